# revision 1
# baseline (speedup 1.0000x reference)
"""Channel self-attention kernel for Trainium2 (Bass/Tile), 8-core data parallel.

Reference computation (per batch b, with q = x[b].reshape(C, H*W)):
    E    = q @ q.T                      # [C, C] gram over n = H*W
    attn = softmax(E, axis=-1)
    out  = gamma * (attn @ q) + x[b]

Key algebraic fold: since q IS x[b] (reshaped), the final elementwise op
folds into the second matmul:
    out = (gamma * attn + I) @ q
so the kernel never needs a separate elementwise add over the full tensor.

Sharding: pure data parallel, batch dim (16) split over 8 cores, 2 batches
per core. gamma replicated. No collectives.

Per-core dataflow (per batch of the core's 2):
  1. q loaded HBM->SBUF in chunks of [128, 3072] f32, resident for the whole
     batch (+2 lookahead pool slots so the next batch's loads overlap this
     batch's second-matmul phase).
  2. Each chunk is PE-transposed 128x128 at a time into PSUM (fp32,
     2 cyc/row); the mandatory PSUM->SBUF evacuation (ACT) doubles as a
     bf16 cast, feeding the E += qT.T @ qT accumulation (PE, bf16 in /
     fp32 PSUM accum). bf16 is safe here: E only feeds a softmax whose
     logits have dynamic range O(n)=36864, while bf16 quantization
     perturbs E by O(1).
  3. Softmax on E (DVE reduce-max + ACT exp with fused row-sum), then
     M = gamma*(attn - I) built in SBUF (DVE), transposed on PE, cast bf16.
  4. Per output chunk: correction = M.T^T @ q_bf16 on PE (result is ~0 --
     attn ~= I for gaussian inputs -- so bf16 costs nothing), then one DVE
     fused op out = (1+gamma)*q + correction in exact fp32, stores of
     [128, 1536] chunks. The main term never leaves fp32.
"""

import os
import sys

for _p in ("/opt/trn_rl_repo", "/root/.axon_site/_ro/trn_rl_repo"):
    if os.path.isdir(_p) and _p not in sys.path:
        sys.path.append(_p)

from contextlib import ExitStack

import numpy as np

import concourse.bacc as bacc
import concourse.bass as bass
import concourse.tile as tile
from concourse import mybir
from concourse.bass_utils import run_bass_kernel_spmd
from concourse.masks import make_identity

# Problem shape (hardcoded; kernel.py must be self-contained).
B, C, H, W = 16, 128, 192, 192
N = H * W                     # 36864
NCORES = 8
BPC = B // NCORES             # 2 batches per core

# Tiling defaults
LOAD_CHUNK = 3072             # cols per load DMA (1.57 MB per transfer)
EXTRA_QBUFS = 2               # lookahead slots for cross-batch prefetch
TGROUP = 512                  # transpose group: one PSUM bank of f32
MM2_N = 512                   # second-matmul moving dim (one f32 PSUM bank)
STORE_CHUNK = 1536            # cols per store DMA (0.79 MB per transfer)
STORE_ENG = "sync"            # HWDGE ring for stores: "scalar" or "sync"
PRECISE = True                # True: out = (1+g)*q + [g*(attn-I)]@q with the
                              #   tiny correction matmul in bf16 and the main
                              #   term an exact fp32 DVE fma. False: single
                              #   float32r matmul out = [g*attn + I]@q (q gets
                              #   rounded to ~12-bit mantissa by the hardware).

F32 = mybir.dt.float32
F32R = mybir.dt.float32r
BF16 = mybir.dt.bfloat16


def build_bass(reps: int = 1, load_chunk: int = LOAD_CHUNK,
               extra_qbufs: int = EXTRA_QBUFS, store_chunk: int = STORE_CHUNK,
               store_eng: str = STORE_ENG, mm2_n: int = MM2_N,
               precise: bool = PRECISE, qt_eng: str = "scalar",
               tdt: str = "f32", qts_bufs: int = 4, psum4: bool = True) -> bass.Bass:
    """reps>1 repeats the whole computation (for wall-clock timing only)."""
    NLOAD = N // load_chunk
    NSTORE = N // store_chunk
    assert N % load_chunk == 0 and N % store_chunk == 0
    assert load_chunk % mm2_n == 0 and store_chunk % mm2_n == 0
    nc = bacc.Bacc("TRN2", target_bir_lowering=False, debug=False)
    # precise: q stays exact fp32 end-to-end. Non-precise: declare x (and its
    # SBUF tiles) float32r so they can feed the f32r matmul directly — the
    # load then rounds q to f32r precision on the way in.
    QDT = F32 if precise else F32R
    x = nc.dram_tensor("x", [BPC, C, N], QDT, kind="ExternalInput")
    gamma = nc.dram_tensor("gamma", [1], F32, kind="ExternalInput")
    out = nc.dram_tensor("out", [BPC, C, N], F32, kind="ExternalOutput")

    with tile.TileContext(nc) as tc, ExitStack() as ctx:
        consts = ctx.enter_context(tc.tile_pool(name="consts", bufs=1))
        pq = ctx.enter_context(tc.tile_pool(name="q", bufs=NLOAD + extra_qbufs))
        pqT = ctx.enter_context(tc.tile_pool(name="qT", bufs=qts_bufs))
        if tdt == "bf16":
            pq16 = ctx.enter_context(tc.tile_pool(name="q16", bufs=2))
        pout = ctx.enter_context(tc.tile_pool(name="outsb", bufs=2))
        if precise:
            prhs = ctx.enter_context(tc.tile_pool(name="rhs16", bufs=2))
        psm = ctx.enter_context(tc.tile_pool(name="smalls", bufs=2))
        ppE = ctx.enter_context(tc.tile_pool(name="psE", bufs=2, space="PSUM"))
        ppT = ctx.enter_context(tc.tile_pool(name="psT", bufs=2, space="PSUM"))
        if psum4:
            # M's transpose shares the qTp slots; the freed bank buys a 4th
            # mm2-output buffer for deeper PE/DVE overlap in phase 3.
            ppM = ppT
            ppO = ctx.enter_context(tc.tile_pool(name="psO", bufs=4, space="PSUM"))
        else:
            ppM = ctx.enter_context(tc.tile_pool(name="psM", bufs=1, space="PSUM"))
            ppO = ctx.enter_context(tc.tile_pool(name="psO", bufs=3, space="PSUM"))

        ident32 = consts.tile([128, 128], F32)
        make_identity(nc, ident32)
        if tdt == "bf16":
            identQ = consts.tile([128, 128], BF16)
            make_identity(nc, identQ)
        elif precise:
            identQ = ident32
        else:
            identQ = consts.tile([128, 128], F32R)   # matmul dtype pairing
            make_identity(nc, identQ)
        gamma_sb = consts.tile([128, 1], F32)
        nc.gpsimd.dma_start(out=gamma_sb, in_=gamma[0:1].to_broadcast((128, 1)))
        if precise:
            gI = consts.tile([128, 128], F32)      # gamma * I
            nc.vector.tensor_scalar_mul(gI, ident32, gamma_sb)
            s1p = consts.tile([128, 1], F32)       # 1 + gamma
            nc.vector.tensor_scalar_add(s1p, gamma_sb, 1.0)

        for b in [b for _ in range(reps) for b in range(BPC)]:
            # ---- Phase 1: load q, build E = q @ q.T in PSUM ----
            E = ppE.tile([128, 128], F32, tag="E")
            q_tiles = []
            n_mm = N // 128  # total accumulating matmuls into E
            mm_i = 0
            for t in range(NLOAD):
                q32 = pq.tile([128, load_chunk], QDT, tag="q32")
                q_tiles.append(q32)
                nc.sync.dma_start(
                    out=q32, in_=x[b, :, t * load_chunk:(t + 1) * load_chunk]
                )
                if tdt == "bf16":
                    tsrc = pq16.tile([128, load_chunk], BF16, tag="q16")
                    nc.vector.tensor_copy(out=tsrc, in_=q32)  # 2x-mode cast
                    tgroup = 2 * TGROUP     # bf16: full bank = 1024 elems
                else:
                    tsrc = q32
                    tgroup = TGROUP
                for s in range(load_chunk // tgroup):
                    # Transpose q on PE; the mandatory PSUM->SBUF evacuation
                    # doubles as the bf16 cast feeding the E matmuls.
                    qTp = ppT.tile([128, tgroup], tsrc.dtype, tag="qTp")
                    for u in range(tgroup // 128):
                        col = s * tgroup + u * 128
                        nc.tensor.transpose(
                            qTp[:, u * 128:(u + 1) * 128],
                            tsrc[:, col:col + 128],
                            identQ,
                        )
                    qTs = pqT.tile([128, tgroup], BF16, tag="qTs")
                    if qt_eng == "scalar":
                        nc.scalar.copy(qTs, qTp)
                    else:
                        nc.vector.tensor_copy(out=qTs, in_=qTp)
                    for u in range(tgroup // 128):
                        nc.tensor.matmul(
                            E,
                            qTs[:, u * 128:(u + 1) * 128],
                            qTs[:, u * 128:(u + 1) * 128],
                            start=(mm_i == 0),
                            stop=(mm_i == n_mm - 1),
                            skip_group_check=True,
                        )
                        mm_i += 1

            # ---- Phase 2: softmax(E) -> M = gamma*attn + I -> M.T ----
            negmax = psm.tile([128, 1], F32, tag="negmax")
            nc.vector.tensor_reduce(
                out=negmax, in_=E, axis=mybir.AxisListType.X,
                op=mybir.AluOpType.max, negate=True,
            )
            P = psm.tile([128, 128], F32, tag="P")
            Z = psm.tile([128, 1], F32, tag="Z")
            nc.scalar.activation(
                P, E, mybir.ActivationFunctionType.Exp,
                bias=negmax, scale=1.0, accum_out=Z,
            )
            rz = psm.tile([128, 1], F32, tag="rz")
            nc.vector.reciprocal(rz, Z)
            s_ap = psm.tile([128, 1], F32, tag="s")
            nc.vector.tensor_mul(s_ap, rz, gamma_sb)       # s = gamma / Z
            M = psm.tile([128, 128], F32, tag="M")
            if precise:
                nc.vector.scalar_tensor_tensor(            # M = gamma*(attn - I)
                    M, P, s_ap, gI,
                    op0=mybir.AluOpType.mult, op1=mybir.AluOpType.subtract,
                )
            else:
                nc.vector.scalar_tensor_tensor(            # M = gamma*attn + I
                    M, P, s_ap, ident32,
                    op0=mybir.AluOpType.mult, op1=mybir.AluOpType.add,
                )
            MTp = ppM.tile([128, 128], F32, tag="qTp" if psum4 else "MTp")
            nc.tensor.transpose(MTp, M, ident32)
            MT = psm.tile([128, 128], BF16 if precise else F32R, tag="MT")
            nc.scalar.copy(MT, MTp)

            # ---- Phase 3: out = M @ q, chunked stores ----
            store_dma = nc.scalar.dma_start if store_eng == "scalar" else nc.sync.dma_start
            for j in range(NSTORE):
                o_sb = pout.tile([128, store_chunk], F32, tag="osb")
                if precise:
                    # bf16 copy of this q span for the correction matmul
                    rhs16 = prhs.tile([128, store_chunk], BF16, tag="rhs16")
                    if store_chunk == load_chunk:
                        nc.scalar.copy(rhs16, q_tiles[j])
                for k in range(store_chunk // mm2_n):
                    col = j * store_chunk + k * mm2_n
                    t_idx, off = divmod(col, load_chunk)
                    op = ppO.tile([128, mm2_n], F32, tag="op")
                    if precise:
                        ks = slice(k * mm2_n, (k + 1) * mm2_n)
                        if store_chunk != load_chunk:
                            nc.scalar.copy(rhs16[:, ks], q_tiles[t_idx][:, off:off + mm2_n])
                        nc.tensor.matmul(op, MT, rhs16[:, ks], start=True, stop=True)
                        # out = (1+gamma)*q + [gamma*(attn-I)]@q, fused on DVE
                        nc.vector.scalar_tensor_tensor(
                            o_sb[:, ks],
                            q_tiles[t_idx][:, off:off + mm2_n],
                            s1p, op,
                            op0=mybir.AluOpType.mult, op1=mybir.AluOpType.add,
                        )
                    else:
                        nc.tensor.matmul(
                            op, MT, q_tiles[t_idx][:, off:off + mm2_n],
                            start=True, stop=True,
                        )
                        nc.scalar.copy(o_sb[:, k * mm2_n:(k + 1) * mm2_n], op)
                store_dma(
                    out=out[b, :, j * store_chunk:(j + 1) * store_chunk],
                    in_=o_sb,
                )

    nc.compile()
    return nc


def kernel_ex(x: np.ndarray, gamma: np.ndarray, **run_kwargs):
    """Run the kernel; returns (out, BassKernelResults)."""
    x = np.ascontiguousarray(np.asarray(x), dtype=np.float32).reshape(B, C, N)
    gamma = np.ascontiguousarray(np.asarray(gamma), dtype=np.float32)
    nc = build_bass()
    in_maps = [
        {"x": np.ascontiguousarray(x[i * BPC:(i + 1) * BPC]), "gamma": gamma}
        for i in range(NCORES)
    ]
    res = run_bass_kernel_spmd(nc, in_maps, core_ids=list(range(NCORES)), **run_kwargs)
    out = np.concatenate([r["out"] for r in res.results], axis=0)
    return out.reshape(B, C, H, W), res


def kernel(x: np.ndarray, gamma: np.ndarray) -> np.ndarray:
    out, _ = kernel_ex(x, gamma)
    return out



# revision 2
# speedup vs baseline: 1.2218x; 1.2218x over previous
"""Channel self-attention kernel for Trainium2 (Bass/Tile), 8-core data parallel.

Reference computation (per batch b, with q = x[b].reshape(C, H*W)):
    E    = q @ q.T                      # [C, C] gram over n = H*W
    attn = softmax(E, axis=-1)
    out  = gamma * (attn @ q) + x[b]

Algebraic fold: since q IS x[b] (reshaped), the elementwise tail folds into
the second matmul:  out = (gamma*attn + I) @ q.

Sharding: pure data parallel, batch dim (16) split over 8 cores, 2 batches
per core. gamma replicated. No collectives.

Per-core design (memory-roofline oriented; HBM traffic = read x once +
write out once = 75.5 MB per core):

  - Loads ride the gpsimd (SWDGE) ring and CAST f32->bf16 in flight, so the
    SBUF-resident q is 2 B/elem and TWO full batches stay resident (qdepth=2).
    The load queue therefore always has a full batch of lookahead and never
    idles during softmax or phase 3. 4.7 MB HBM reads per transfer.
  - Stores ride the sync (SP HWDGE) ring: 2.36 MB f32 writes. Loads and
    stores sit on different queues; the 16 SDMA engines round-robin between
    them, keeping HBM busy in both directions. Neither queue is ever blocked
    behind a compute engine (SP and Pool issue no compute here) -- issuing
    DMAs from ACT/DVE queues would serialize them behind compute waits.
  - Phase 1 (per 9216-col chunk): PE transposes q16 128x128 at a time into
    PSUM (bf16, 1 cyc/row); ACT evacuates PSUM->SBUF (ACT is otherwise idle
    in phase 1); PE accumulates E += qT.T @ qT (bf16 in / fp32 PSUM accum).
  - Phase 2: softmax via DVE reduce-max + ACT exp (fused row-sum) + DVE
    reciprocal; M = gamma*attn + I built on DVE; PE-transposed; cast bf16.
  - Phase 3: out = M.T.T @ q16 on PE in 512-col slices; DVE (idle in phase 3)
    copies PSUM->SBUF f32; SP stores 4608-col chunks. Splitting evac=ACT /
    out-copy=DVE keeps either engine from pacing the store stream (~12%
    stall when ACT did both).

Accuracy: bf16 rounding of q and of the (1+gamma) diagonal of M gives
rel err ~4e-3 vs the 2e-2 gate. E is bf16-input/fp32-accum; its softmax is
insensitive (logit gaps are O(n)).
"""

import os
import sys

for _p in ("/opt/trn_rl_repo", "/root/.axon_site/_ro/trn_rl_repo"):
    if os.path.isdir(_p) and _p not in sys.path:
        sys.path.append(_p)

from contextlib import ExitStack

import numpy as np

import concourse.bacc as bacc
import concourse.bass as bass
import concourse.tile as tile
from concourse import mybir
from concourse.bass_utils import run_bass_kernel_spmd
from concourse.masks import make_identity

# Problem shape (hardcoded; kernel.py must be self-contained).
B, C, H, W = 16, 128, 192, 192
N = H * W                     # 36864
NCORES = 8
BPC = B // NCORES             # 2 batches per core

LOAD_CHUNK = 9216             # f32 HBM read 4.7 MB; bf16 dest 18 KB/partition
STORE_CHUNK = 6144            # f32 HBM write 3.15 MB
TGROUP = 1024                 # transpose group: one PSUM bank of bf16
MM2_N = 512                   # second-matmul moving dim (one f32 PSUM bank)
QDEPTH = 2                    # batches of q16 kept resident (lookahead)

F32 = mybir.dt.float32
BF16 = mybir.dt.bfloat16


def build_bass(reps: int = 1, load_chunk: int = LOAD_CHUNK,
               store_chunk: int = STORE_CHUNK, mm2_n: int = MM2_N,
               tgroup: int = TGROUP, qdepth: int = QDEPTH) -> bass.Bass:
    """reps>1 repeats the whole computation (for wall-clock timing only)."""
    NLOAD = N // load_chunk
    NSTORE = N // store_chunk
    assert N % load_chunk == 0 and N % store_chunk == 0
    assert load_chunk % tgroup == 0 and tgroup % 128 == 0
    assert load_chunk % mm2_n == 0 and store_chunk % mm2_n == 0
    nc = bacc.Bacc("TRN2", target_bir_lowering=False, debug=False)
    x = nc.dram_tensor("x", [BPC, C, N], F32, kind="ExternalInput")
    gamma = nc.dram_tensor("gamma", [1], F32, kind="ExternalInput")
    out = nc.dram_tensor("out", [BPC, C, N], F32, kind="ExternalOutput")

    with tile.TileContext(nc) as tc, ExitStack() as ctx:
        consts = ctx.enter_context(tc.tile_pool(name="consts", bufs=1))
        pq16 = ctx.enter_context(tc.tile_pool(name="q16", bufs=qdepth * NLOAD))
        pqT = ctx.enter_context(tc.tile_pool(name="qT", bufs=4))
        pout = ctx.enter_context(tc.tile_pool(name="outsb", bufs=2))
        psm = ctx.enter_context(tc.tile_pool(name="smalls", bufs=2))
        ppE = ctx.enter_context(tc.tile_pool(name="psE", bufs=2, space="PSUM"))
        ppT = ctx.enter_context(tc.tile_pool(name="psT", bufs=2, space="PSUM"))
        ppO = ctx.enter_context(tc.tile_pool(
            name="psO", bufs=4 if mm2_n <= 512 else 2, space="PSUM"))

        identB = consts.tile([128, 128], BF16)
        make_identity(nc, identB)
        ident32 = consts.tile([128, 128], F32)
        make_identity(nc, ident32)
        gamma_sb = consts.tile([128, 1], F32)
        nc.gpsimd.dma_start(out=gamma_sb, in_=gamma[0:1].to_broadcast((128, 1)))

        for b in [b for _ in range(reps) for b in range(BPC)]:
            # ---- Phase 1: cast-load q16, build E = q16 @ q16.T in PSUM ----
            E = ppE.tile([128, 128], F32, tag="E")
            q_tiles = []
            n_mm = N // 128
            mm_i = 0
            for t in range(NLOAD):
                q16 = pq16.tile([128, load_chunk], BF16, tag="q16")
                q_tiles.append(q16)
                nc.gpsimd.dma_start(
                    out=q16, in_=x[b, :, t * load_chunk:(t + 1) * load_chunk])
                for s in range(load_chunk // tgroup):
                    qTp = ppT.tile([128, tgroup], BF16, tag="qTp")
                    for u in range(tgroup // 128):
                        col = s * tgroup + u * 128
                        nc.tensor.transpose(
                            qTp[:, u * 128:(u + 1) * 128],
                            q16[:, col:col + 128],
                            identB,
                        )
                    qTs = pqT.tile([128, tgroup], BF16, tag="qTs")
                    nc.scalar.copy(qTs, qTp)
                    for u in range(tgroup // 128):
                        nc.tensor.matmul(
                            E,
                            qTs[:, u * 128:(u + 1) * 128],
                            qTs[:, u * 128:(u + 1) * 128],
                            start=(mm_i == 0),
                            stop=(mm_i == n_mm - 1),
                            skip_group_check=True,
                        )
                        mm_i += 1

            # ---- Phase 2: softmax(E) -> M = gamma*attn + I -> M.T bf16 ----
            negmax = psm.tile([128, 1], F32, tag="negmax")
            nc.vector.tensor_reduce(
                out=negmax, in_=E, axis=mybir.AxisListType.X,
                op=mybir.AluOpType.max, negate=True,
            )
            P = psm.tile([128, 128], F32, tag="P")
            Z = psm.tile([128, 1], F32, tag="Z")
            nc.scalar.activation(
                P, E, mybir.ActivationFunctionType.Exp,
                bias=negmax, scale=1.0, accum_out=Z,
            )
            rz = psm.tile([128, 1], F32, tag="rz")
            nc.vector.reciprocal(rz, Z)
            s_ap = psm.tile([128, 1], F32, tag="s")
            nc.vector.tensor_mul(s_ap, rz, gamma_sb)       # s = gamma / Z
            M = psm.tile([128, 128], F32, tag="M")
            nc.vector.scalar_tensor_tensor(                # M = gamma*attn + I
                M, P, s_ap, ident32,
                op0=mybir.AluOpType.mult, op1=mybir.AluOpType.add,
            )
            MTp = ppT.tile([128, 128], F32, tag="qTp")
            nc.tensor.transpose(MTp, M, ident32)
            MT = psm.tile([128, 128], BF16, tag="MT")
            nc.scalar.copy(MT, MTp)                        # cast f32 -> bf16

            # ---- Phase 3: out = M @ q16, chunked stores ----
            for j in range(NSTORE):
                o_sb = pout.tile([128, store_chunk], F32, tag="osb")
                for k in range(store_chunk // mm2_n):
                    col = j * store_chunk + k * mm2_n
                    t_idx, off = divmod(col, load_chunk)
                    op = ppO.tile([128, mm2_n], F32, tag="op")
                    nc.tensor.matmul(
                        op, MT, q_tiles[t_idx][:, off:off + mm2_n],
                        start=True, stop=True,
                    )
                    nc.vector.tensor_copy(
                        out=o_sb[:, k * mm2_n:(k + 1) * mm2_n], in_=op)
                nc.sync.dma_start(
                    out=out[b, :, j * store_chunk:(j + 1) * store_chunk],
                    in_=o_sb,
                )

    nc.compile()
    return nc


def kernel_ex(x: np.ndarray, gamma: np.ndarray, **run_kwargs):
    """Run the kernel; returns (out, BassKernelResults)."""
    x = np.ascontiguousarray(np.asarray(x), dtype=np.float32).reshape(B, C, N)
    gamma = np.ascontiguousarray(np.asarray(gamma), dtype=np.float32)
    nc = build_bass()
    in_maps = [
        {"x": np.ascontiguousarray(x[i * BPC:(i + 1) * BPC]), "gamma": gamma}
        for i in range(NCORES)
    ]
    res = run_bass_kernel_spmd(nc, in_maps, core_ids=list(range(NCORES)), **run_kwargs)
    out = np.concatenate([r["out"] for r in res.results], axis=0)
    return out.reshape(B, C, H, W), res


def kernel(x: np.ndarray, gamma: np.ndarray) -> np.ndarray:
    out, _ = kernel_ex(x, gamma)
    return out
